# revision 16
# baseline (speedup 1.0000x reference)
"""Trainium2 Bass kernel for the GRU-decoder-with-Luong-attention module.

Problem shapes (hardcoded per the grading contract):
  B=64 batch, L=32 decode steps, T=2048 encoder positions,
  V=32000 vocab, E=H=128.

Sharding: data-parallel over batch across 8 NeuronCores (8 batches/core).
Small params are replicated; fc_k is streamed from HBM on every core.
enc_output is additionally passed in transposed layout (encT) so the score
matmul can contract over H without on-chip transposes.

Key algebraic restructuring (exact up to f32 rounding):
  * The scan carry is only the GRU hidden state h; attention/ctx/logits do
    not feed back. So we run the (tiny) 32-step GRU recurrence first and
    then do attention for all 32 steps with batched matmuls.
  * The GRU input projections x@W (teacher-forced tokens, known upfront)
    are hoisted out of the recurrence into three batched matmuls.
  * enc_feat = enc @ encW + encW_b is never materialized:
      - score[l,t] = (dec_feat[l]*V) . enc_feat[t] = wproj[l] . enc[t] + c[l]
        with wproj = (dec_feat*V) @ encW^T; c[l] is constant over t so it
        cancels in softmax (as does V_b).
      - ctx[l] = sum_t attn[l,t] enc_feat[t] = (sum_t attn[l,t] enc[t]) @ encW
        + encW_b   (softmax rows sum to 1).
"""

import numpy as np

B, L, T, V, E, H = 64, 32, 2048, 32000, 128, 128
NC_N = 8          # cores
BC = B // NC_N    # batches per core = 8
SOS_ID = 0
ROWS = BC * L     # 256 output rows per core
FC_CH = 1600      # fc_k columns per SBUF buffer (4 psum chunks of 400)
FC_NB = V // FC_CH  # 20

_cache = {}


def _build_nc(use_mask: bool, use_fcb: bool):
    import concourse.bass as bass
    import concourse.mybir as mybir
    import concourse.tile as tile
    from concourse import bacc
    from concourse.masks import make_identity

    fp32 = mybir.dt.float32
    AF = mybir.ActivationFunctionType
    ALU = mybir.AluOpType
    AX = mybir.AxisListType

    nc = bacc.Bacc()

    # ---- DRAM I/O ------------------------------------------------------
    enc_in = nc.declare_dram_parameter("enc_in", [BC, T, H], fp32, isOutput=False)
    encT_in = nc.declare_dram_parameter("encT_in", [BC, H, T], fp32, isOutput=False)
    toks_in = nc.declare_dram_parameter("toks", [128, 2], mybir.dt.int32, isOutput=False)
    embed_in = nc.declare_dram_parameter("embed", [V, E], fp32, isOutput=False)
    gate_k_in = nc.declare_dram_parameter("gate_k", [2 * H, 2 * H], fp32, isOutput=False)
    gate_b_in = nc.declare_dram_parameter("gate_b", [2 * H, 1], fp32, isOutput=False)
    cand_k_in = nc.declare_dram_parameter("cand_k", [2 * H, H], fp32, isOutput=False)
    cand_b_in = nc.declare_dram_parameter("cand_b", [H, 1], fp32, isOutput=False)
    dwk_in = nc.declare_dram_parameter("decW_k", [H, H], fp32, isOutput=False)
    dwb_in = nc.declare_dram_parameter("decW_b", [H, 1], fp32, isOutput=False)
    ewk_in = nc.declare_dram_parameter("encW_k", [H, H], fp32, isOutput=False)
    ewkT_in = nc.declare_dram_parameter("encW_kT", [H, H], fp32, isOutput=False)
    ewb_in = nc.declare_dram_parameter("encW_b", [H, 1], fp32, isOutput=False)
    vk_in = nc.declare_dram_parameter("V_k", [H, 1], fp32, isOutput=False)
    fck_in = nc.declare_dram_parameter("fc_k", [H, V], fp32, isOutput=False)
    if use_fcb:
        fcb_in = nc.declare_dram_parameter("fc_b", [1, V], fp32, isOutput=False)
    if use_mask:
        maskb_in = nc.declare_dram_parameter("maskb", [1, BC * T], fp32, isOutput=False)

    logits_out = nc.declare_dram_parameter("logits", [ROWS, V], fp32, isOutput=True)
    attns_out = nc.declare_dram_parameter("attns", [ROWS, T], fp32, isOutput=True)

    with tile.TileContext(nc) as tc, tc.tile_pool(name="singles", bufs=1) as singles, \
         tc.tile_pool(name="enc_pool", bufs=1) as enc_pool, \
         tc.tile_pool(name="gru_pool", bufs=3) as gru_pool, \
         tc.tile_pool(name="attn_pool", bufs=2) as attn_pool, \
         tc.tile_pool(name="attnT_pool", bufs=1) as attnT_pool, \
         tc.tile_pool(name="fck_pool", bufs=3) as fck_pool, \
         tc.tile_pool(name="lg_pool", bufs=3) as lg_pool, \
         tc.tile_pool(name="ps1", bufs=4, space="PSUM") as ps1, \
         tc.tile_pool(name="ps_score", bufs=1, space="PSUM") as ps_score:

        # ---- load constants/params ------------------------------------
        ident_g = singles.tile([128, 128], fp32)
        make_identity(nc, ident_g)
        ident = singles.tile([128, 128], fp32)
        nc.vector.tensor_copy(out=ident, in_=ident_g)
        ones_row = singles.tile([1, 128], fp32)
        nc.vector.memset(ones_row, 1.0)

        gkx = singles.tile([128, 256], fp32)   # gate_k rows 0:128  (x part)
        nc.sync.dma_start(out=gkx, in_=gate_k_in[0:128, :])
        gkh = singles.tile([128, 256], fp32)   # gate_k rows 128:256 (h part)
        nc.sync.dma_start(out=gkh, in_=gate_k_in[128:256, :])
        ckx = singles.tile([128, 128], fp32)
        nc.sync.dma_start(out=ckx, in_=cand_k_in[0:128, :])
        ckh = singles.tile([128, 128], fp32)
        nc.sync.dma_start(out=ckh, in_=cand_k_in[128:256, :])
        dwk = singles.tile([128, 128], fp32)
        nc.sync.dma_start(out=dwk, in_=dwk_in[:, :])
        ewk = singles.tile([128, 128], fp32)
        nc.sync.dma_start(out=ewk, in_=ewk_in[:, :])
        ewkT = singles.tile([128, 128], fp32)
        nc.sync.dma_start(out=ewkT, in_=ewkT_in[:, :])

        gb0 = singles.tile([128, 1], fp32)
        nc.sync.dma_start(out=gb0, in_=gate_b_in[0:128, :])
        gb1 = singles.tile([128, 1], fp32)
        nc.sync.dma_start(out=gb1, in_=gate_b_in[128:256, :])
        cb = singles.tile([128, 1], fp32)
        nc.sync.dma_start(out=cb, in_=cand_b_in[:, :])
        dwb = singles.tile([128, 1], fp32)
        nc.sync.dma_start(out=dwb, in_=dwb_in[:, :])
        ewb = singles.tile([128, 1], fp32)
        nc.sync.dma_start(out=ewb, in_=ewb_in[:, :])
        vkT = singles.tile([128, 1], fp32)
        nc.sync.dma_start(out=vkT, in_=vk_in[:, :])

        if use_fcb:
            fcb_sb = singles.tile([1, V], fp32)
            nc.sync.dma_start(out=fcb_sb, in_=fcb_in[:, :])
        if use_mask:
            maskb_sb = singles.tile([1, BC * T], fp32)
            nc.sync.dma_start(out=maskb_sb, in_=maskb_in[:, :])

        toks_sb = singles.tile([128, 2], mybir.dt.int32)
        nc.sync.dma_start(out=toks_sb, in_=toks_in[:, :])

        # ---- enc loads: raw [t,h] tiles and transposed [h,t] ------------
        enc_sb, encT_sb = [], []
        for b in range(BC):
            e_t = enc_pool.tile([128, 16, 128], fp32, name=f"enc_{b}", tag=f"enc_{b}")
            nc.sync.dma_start(out=e_t, in_=enc_in[b].rearrange("(k p) h -> p k h", p=128))
            enc_sb.append(e_t)
            eT = enc_pool.tile([128, T], fp32, name=f"encT_{b}", tag=f"encT_{b}")
            nc.sync.dma_start(out=eT, in_=encT_in[b])
            encT_sb.append(eT)

        # ---- embedding gather + transpose -> xT [E, (l,b)] -------------
        xT = singles.tile([128, L * BC], fp32)      # columns l*8+b, local batches
        for g in range(2):
            x_g = gru_pool.tile([128, 128], fp32, tag="x_g")
            nc.gpsimd.indirect_dma_start(
                out=x_g,
                out_offset=None,
                in_=embed_in[:, :],
                in_offset=bass.IndirectOffsetOnAxis(ap=toks_sb[:, g:g + 1], axis=0),
            )
            x_g2 = gru_pool.tile([128, 128], fp32, tag="x_g2")
            nc.vector.tensor_copy(out=x_g2, in_=x_g)
            xg_ps = ps1.tile([128, 128], fp32, tag="ps1")
            nc.tensor.transpose(out=xg_ps, in_=x_g2, identity=ident)
            nc.vector.tensor_copy(out=xT[:, g * 128:(g + 1) * 128], in_=xg_ps)

        # ---- GRU recurrence over L steps (local batches) ----------------
        # x-projections accumulate directly in PSUM (no h dependence, so the
        # scheduler runs them ahead); biases applied on the ACT engine.
        HallT = singles.tile([128, L * BC], fp32)   # h_l^T columns l*8+b
        h0 = singles.tile([128, BC], fp32)
        nc.vector.memset(h0, 0.0)
        hT = h0
        for l in range(L):
            xs = xT[:, l * BC:(l + 1) * BC]
            g_ps = ps1.tile([128, 2 * BC], fp32, tag="ps1", name=f"g_ps{l}")
            nc.tensor.matmul(out=g_ps[:, 0:BC], lhsT=gkx[:, 0:128], rhs=xs, start=True, stop=False)
            nc.tensor.matmul(out=g_ps[:, 0:BC], lhsT=gkh[:, 0:128], rhs=hT, start=False, stop=True)
            nc.tensor.matmul(out=g_ps[:, BC:2 * BC], lhsT=gkx[:, 128:256], rhs=xs, start=True, stop=False)
            nc.tensor.matmul(out=g_ps[:, BC:2 * BC], lhsT=gkh[:, 128:256], rhs=hT, start=False, stop=True)
            r_sb = gru_pool.tile([128, BC], fp32, tag="r_sb")
            u_sb = gru_pool.tile([128, BC], fp32, tag="u_sb")
            nc.scalar.activation(out=r_sb, in_=g_ps[:, 0:BC], func=AF.Sigmoid, bias=gb0)
            nc.scalar.activation(out=u_sb, in_=g_ps[:, BC:2 * BC], func=AF.Sigmoid, bias=gb1)
            rh = gru_pool.tile([128, BC], fp32, tag="rh")
            nc.vector.tensor_mul(out=rh, in0=r_sb, in1=hT)
            c_ps = ps1.tile([128, BC], fp32, tag="ps1", name=f"c_ps{l}")
            nc.tensor.matmul(out=c_ps, lhsT=ckx, rhs=xs, start=True, stop=False)
            nc.tensor.matmul(out=c_ps, lhsT=ckh, rhs=rh, start=False, stop=True)
            c_sb = gru_pool.tile([128, BC], fp32, tag="c_sb")
            nc.scalar.activation(out=c_sb, in_=c_ps, func=AF.Tanh, bias=cb)
            d_sb = gru_pool.tile([128, BC], fp32, tag="d_sb")
            nc.vector.tensor_sub(out=d_sb, in0=hT, in1=c_sb)        # h - c
            nc.vector.tensor_mul(out=d_sb, in0=u_sb, in1=d_sb)
            hcol = HallT[:, l * BC:(l + 1) * BC]
            nc.vector.tensor_add(out=hcol, in0=c_sb, in1=d_sb)      # c + u*(h-c)
            hT = hcol

        Hall3 = HallT.rearrange("p (l b) -> p l b", b=BC)

        # ---- attention, two groups of 4 batches -------------------------
        # wproj for all 8 batches in one batched chain:
        #   wpT_all[h', (l,b)] = encW_k @ ((dec_feat + decW_b) * V_k)^T
        dec_ps = ps1.tile([128, 256], fp32, tag="ps1")
        nc.tensor.matmul(out=dec_ps, lhsT=dwk, rhs=HallT, start=True, stop=True)
        wT_all = singles.tile([128, 256], fp32)
        nc.vector.tensor_scalar(out=wT_all, in0=dec_ps, scalar1=dwb, scalar2=vkT,
                                op0=ALU.add, op1=ALU.mult)
        wp_ps = ps1.tile([128, 256], fp32, tag="ps1")
        nc.tensor.matmul(out=wp_ps, lhsT=ewkT, rhs=wT_all, start=True, stop=True)
        wpT_all = singles.tile([128, 256], fp32)
        nc.vector.tensor_copy(out=wpT_all, in_=wp_ps)
        wpT3 = wpT_all.rearrange("p (l b) -> p l b", b=BC)

        ctxT_all = singles.tile([128, ROWS], fp32)    # columns b_local*32+l
        for g01 in range(2):
            sc_ps = ps_score.tile([128, T], fp32, tag="score", name=f"sc{g01}")
            mx4 = gru_pool.tile([128, 4], fp32, tag="mx4")
            for c in range(4):
                for j in range(4):
                    b = g01 * 4 + j
                    nc.tensor.matmul(out=sc_ps[32 * j:32 * (j + 1), c * 512:(c + 1) * 512],
                                     lhsT=wpT3[:, :, b], rhs=encT_sb[b][:, c * 512:(c + 1) * 512],
                                     start=True, stop=not use_mask,
                                     tile_position=(0, 32 * j))
                    if use_mask:
                        nc.tensor.matmul(out=sc_ps[32 * j:32 * (j + 1), c * 512:(c + 1) * 512],
                                         lhsT=ones_row[:, 0:32],
                                         rhs=maskb_sb[:, b * T + c * 512: b * T + (c + 1) * 512],
                                         start=False, stop=True,
                                         tile_position=(0, 32 * j))
                nc.vector.tensor_reduce(out=mx4[:, c:c + 1], in_=sc_ps[:, c * 512:(c + 1) * 512],
                                        axis=AX.X, op=ALU.max)
            nmx = gru_pool.tile([128, 1], fp32, tag="nmx")
            nc.vector.tensor_reduce(out=nmx, in_=mx4, axis=AX.X, op=ALU.max, negate=True)
            attn_sb = attn_pool.tile([128, T], fp32, tag="attn")
            sume = gru_pool.tile([128, 1], fp32, tag="sume")
            nc.scalar.activation(out=attn_sb, in_=sc_ps, func=AF.Exp, bias=nmx,
                                 accum_out=sume)
            rs = gru_pool.tile([128, 1], fp32, tag="rs")
            nc.vector.reciprocal(out=rs, in_=sume)
            nc.vector.tensor_scalar_mul(attn_sb, attn_sb, rs)
            nc.sync.dma_start(out=attns_out[g01 * 128:(g01 + 1) * 128, :], in_=attn_sb)

            # transpose attn -> [T, (4b,32l)] tiles
            attnT = attnT_pool.tile([128, 16, 128], fp32, tag="attnT")
            for k in range(16):
                at_ps = ps1.tile([128, 128], fp32, tag="ps1")
                nc.tensor.transpose(out=at_ps, in_=attn_sb[:, k * 128:(k + 1) * 128], identity=ident)
                nc.vector.tensor_copy(out=attnT[:, k, :], in_=at_ps)

            # ctx_raw accumulation over t tiles
            cx2_ps = ps1.tile([128, 128], fp32, tag="ps1", name=f"cx2_{g01}")
            for j in range(4):
                b = g01 * 4 + j
                for k in range(16):
                    nc.tensor.matmul(out=cx2_ps[32 * j:32 * (j + 1), :],
                                     lhsT=attnT[:, k, 32 * j:32 * (j + 1)],
                                     rhs=enc_sb[b][:, k, :],
                                     start=(k == 0), stop=(k == 15),
                                     tile_position=(0, 32 * j))
            cxr = gru_pool.tile([128, 128], fp32, tag="cxr")
            nc.vector.tensor_copy(out=cxr, in_=cx2_ps)
            cxT_ps = ps1.tile([128, 128], fp32, tag="ps1", name=f"cxT{g01}")
            nc.tensor.transpose(out=cxT_ps, in_=cxr, identity=ident)
            cxrT = gru_pool.tile([128, 128], fp32, tag="cxrT")
            nc.vector.tensor_copy(out=cxrT, in_=cxT_ps)
            cf_ps = ps1.tile([128, 128], fp32, tag="ps1", name=f"cf{g01}")
            nc.tensor.matmul(out=cf_ps, lhsT=ewk, rhs=cxrT, start=True, stop=True)
            nc.vector.tensor_scalar_add(ctxT_all[:, g01 * 128:(g01 + 1) * 128], cf_ps, ewb)

        # ---- fc to vocab -------------------------------------------------
        for fi in range(FC_NB):
            fck_sb = fck_pool.tile([128, FC_CH], fp32, tag="fck")
            nc.sync.dma_start(out=fck_sb, in_=fck_in[:, fi * FC_CH:(fi + 1) * FC_CH])
            for mi in range(2):
                lg_sb = lg_pool.tile([128, FC_CH], fp32, tag="lg")
                for cc in range(4):
                    n0 = cc * 400
                    fc_ps = ps1.tile([128, 400], fp32, tag="ps1", name=f"fc{fi}_{mi}_{cc}")
                    nc.tensor.matmul(out=fc_ps, lhsT=ctxT_all[:, mi * 128:(mi + 1) * 128],
                                     rhs=fck_sb[:, n0:n0 + 400], start=True, stop=not use_fcb)
                    if use_fcb:
                        nc.tensor.matmul(out=fc_ps, lhsT=ones_row,
                                         rhs=fcb_sb[:, fi * FC_CH + n0: fi * FC_CH + n0 + 400],
                                         start=False, stop=True)
                    nc.vector.tensor_copy(out=lg_sb[:, n0:n0 + 400], in_=fc_ps)
                nc.sync.dma_start(out=logits_out[mi * 128:(mi + 1) * 128, fi * FC_CH:(fi + 1) * FC_CH],
                                  in_=lg_sb)

    nc.finalize()   # Bacc.compile(): splits multi-waits via event semaphores
    return nc


def _get_nc(use_mask: bool, use_fcb: bool):
    key = (use_mask, use_fcb)
    if key not in _cache:
        _cache[key] = _build_nc(use_mask, use_fcb)
    return _cache[key]


def _host_prep(inputs):
    ti = np.asarray(inputs["target_input"])
    toks = np.concatenate(
        [np.full((B, 1), SOS_ID, np.int64), ti[:, :-1].astype(np.int64)], axis=1)
    mask = np.asarray(inputs["weight_mask"]).astype(np.float32)
    maskb = (mask - 1.0) * 1e30                        # 0 valid / -1e30 masked
    return toks, maskb


def kernel(**inputs):
    return _run(inputs, trace=False)[0]


def _run(inputs, trace=False):
    from concourse.bass_utils import run_bass_kernel_spmd

    toks, maskb = _host_prep(inputs)
    use_mask = bool(np.any(maskb != 0.0))
    fc_b = np.asarray(inputs["fc_b"], np.float32)
    use_fcb = bool(np.any(fc_b != 0.0))

    nc = _get_nc(use_mask, use_fcb)

    enc = np.ascontiguousarray(np.asarray(inputs["enc_output"], np.float32))
    encT = np.ascontiguousarray(enc.transpose(0, 2, 1))
    rep = dict(
        embed=np.asarray(inputs["embed"], np.float32),
        gate_k=np.asarray(inputs["gate_k"], np.float32),
        gate_b=np.asarray(inputs["gate_b"], np.float32).reshape(2 * H, 1),
        cand_k=np.asarray(inputs["cand_k"], np.float32),
        cand_b=np.asarray(inputs["cand_b"], np.float32).reshape(H, 1),
        decW_k=np.asarray(inputs["decW_k"], np.float32),
        decW_b=np.asarray(inputs["decW_b"], np.float32).reshape(H, 1),
        encW_k=np.asarray(inputs["encW_k"], np.float32),
        encW_kT=np.ascontiguousarray(np.asarray(inputs["encW_k"], np.float32).T),
        encW_b=np.asarray(inputs["encW_b"], np.float32).reshape(H, 1),
        V_k=np.asarray(inputs["V_k"], np.float32).reshape(H, 1),
        fc_k=np.asarray(inputs["fc_k"], np.float32),
    )
    if use_fcb:
        rep["fc_b"] = fc_b.reshape(1, V)

    in_maps = []
    for ci in range(NC_N):
        m = dict(rep)
        m["enc_in"] = enc[ci * BC:(ci + 1) * BC]
        m["encT_in"] = encT[ci * BC:(ci + 1) * BC]
        # rows r = l*BC + b_local, stored [128, 2] with r = g*128 + p
        tk = toks[ci * BC:(ci + 1) * BC]               # [BC, L]
        tk_lb = tk.T.reshape(-1)                       # (l, b) order
        m["toks"] = np.ascontiguousarray(tk_lb.reshape(2, 128).T.astype(np.int32))
        if use_mask:
            m["maskb"] = np.ascontiguousarray(
                maskb[ci * BC:(ci + 1) * BC].reshape(1, BC * T))
        in_maps.append(m)

    res = run_bass_kernel_spmd(nc, in_maps, list(range(NC_N)), trace=trace)
    logits = np.concatenate([r["logits"].reshape(BC, L, V) for r in res.results], axis=0)
    attns = np.concatenate([r["attns"].reshape(BC, L, T) for r in res.results], axis=0)
    return (logits, attns), res


# revision 19
# speedup vs baseline: 1.1236x; 1.1236x over previous
"""Trainium2 Bass kernel for the GRU-decoder-with-Luong-attention module.

Problem shapes (hardcoded per the grading contract):
  B=64 batch, L=32 decode steps, T=2048 encoder positions,
  V=32000 vocab, E=H=128.

Sharding: data-parallel over batch across 8 NeuronCores (8 batches/core).
Small params are replicated; fc_k is streamed from HBM on every core.
enc_output is additionally passed in transposed layout (encT) so the score
matmul can contract over H without on-chip transposes.

Key algebraic restructuring (exact up to f32 rounding):
  * The scan carry is only the GRU hidden state h; attention/ctx/logits do
    not feed back. So we run the (tiny) 32-step GRU recurrence first and
    then do attention for all 32 steps with batched matmuls.
  * The GRU input projections x@W (teacher-forced tokens, known upfront)
    are hoisted out of the recurrence into three batched matmuls.
  * enc_feat = enc @ encW + encW_b is never materialized:
      - score[l,t] = (dec_feat[l]*V) . enc_feat[t] = wproj[l] . enc[t] + c[l]
        with wproj = (dec_feat*V) @ encW^T; c[l] is constant over t so it
        cancels in softmax (as does V_b).
      - ctx[l] = sum_t attn[l,t] enc_feat[t] = (sum_t attn[l,t] enc[t]) @ encW
        + encW_b   (softmax rows sum to 1).
"""

import numpy as np

B, L, T, V, E, H = 64, 32, 2048, 32000, 128, 128
NC_N = 8          # cores
BC = B // NC_N    # batches per core = 8
SOS_ID = 0
ROWS = BC * L     # 256 output rows per core
FC_CH = 1600      # fc_k columns per SBUF buffer (4 psum chunks of 400)
FC_NB = V // FC_CH  # 20

_cache = {}


def _build_nc(use_mask: bool, use_fcb: bool):
    import concourse.bass as bass
    import concourse.mybir as mybir
    import concourse.tile as tile
    from concourse import bacc
    from concourse.masks import make_identity

    fp32 = mybir.dt.float32
    AF = mybir.ActivationFunctionType
    ALU = mybir.AluOpType
    AX = mybir.AxisListType
    f32r = mybir.dt.float32r

    nc = bacc.Bacc()

    # ---- DRAM I/O ------------------------------------------------------
    enc_in = nc.declare_dram_parameter("enc_in", [BC, T, H], fp32, isOutput=False)
    encT_in = nc.declare_dram_parameter("encT_in", [BC, H, T], fp32, isOutput=False)
    toks_in = nc.declare_dram_parameter("toks", [128, 2], mybir.dt.int32, isOutput=False)
    embed_in = nc.declare_dram_parameter("embed", [V, E], fp32, isOutput=False)
    gate_k_in = nc.declare_dram_parameter("gate_k", [2 * H, 2 * H], fp32, isOutput=False)
    gate_b_in = nc.declare_dram_parameter("gate_b", [2 * H, 1], fp32, isOutput=False)
    cand_k_in = nc.declare_dram_parameter("cand_k", [2 * H, H], fp32, isOutput=False)
    cand_b_in = nc.declare_dram_parameter("cand_b", [H, 1], fp32, isOutput=False)
    dwk_in = nc.declare_dram_parameter("decW_k", [H, H], fp32, isOutput=False)
    dwb_in = nc.declare_dram_parameter("decW_b", [H, 1], fp32, isOutput=False)
    ewk_in = nc.declare_dram_parameter("encW_k", [H, H], fp32, isOutput=False)
    ewkT_in = nc.declare_dram_parameter("encW_kT", [H, H], fp32, isOutput=False)
    ewb_in = nc.declare_dram_parameter("encW_b", [H, 1], fp32, isOutput=False)
    vk_in = nc.declare_dram_parameter("V_k", [H, 1], fp32, isOutput=False)
    fck_in = nc.declare_dram_parameter("fc_k", [H, V], fp32, isOutput=False)
    if use_fcb:
        fcb_in = nc.declare_dram_parameter("fc_b", [1, V], fp32, isOutput=False)
    if use_mask:
        maskb_in = nc.declare_dram_parameter("maskb", [1, BC * T], fp32, isOutput=False)

    logits_out = nc.declare_dram_parameter("logits", [ROWS, V], fp32, isOutput=True)
    attns_out = nc.declare_dram_parameter("attns", [ROWS, T], fp32, isOutput=True)

    with tile.TileContext(nc) as tc, tc.tile_pool(name="singles", bufs=1) as singles, \
         tc.tile_pool(name="enc_pool", bufs=1) as enc_pool, \
         tc.tile_pool(name="gru_pool", bufs=3) as gru_pool, \
         tc.tile_pool(name="attn_pool", bufs=2) as attn_pool, \
         tc.tile_pool(name="attnT_pool", bufs=1) as attnT_pool, \
         tc.tile_pool(name="fck_pool", bufs=3) as fck_pool, \
         tc.tile_pool(name="lg_pool", bufs=3) as lg_pool, \
         tc.tile_pool(name="ps1", bufs=4, space="PSUM") as ps1, \
         tc.tile_pool(name="ps_score", bufs=1, space="PSUM") as ps_score:

        # ---- load constants/params ------------------------------------
        ident_g = singles.tile([128, 128], fp32)
        make_identity(nc, ident_g)
        ident = singles.tile([128, 128], fp32)
        nc.vector.tensor_copy(out=ident, in_=ident_g)
        ones_row = singles.tile([1, 128], fp32)
        nc.vector.memset(ones_row, 1.0)

        gkx = singles.tile([128, 256], f32r)   # gate_k rows 0:128  (x part)
        nc.sync.dma_start(out=gkx, in_=gate_k_in[0:128, :].bitcast(f32r))
        gkh = singles.tile([128, 256], f32r)   # gate_k rows 128:256 (h part)
        nc.sync.dma_start(out=gkh, in_=gate_k_in[128:256, :].bitcast(f32r))
        ckx = singles.tile([128, 128], f32r)
        nc.sync.dma_start(out=ckx, in_=cand_k_in[0:128, :].bitcast(f32r))
        ckh = singles.tile([128, 128], f32r)
        nc.sync.dma_start(out=ckh, in_=cand_k_in[128:256, :].bitcast(f32r))
        dwk = singles.tile([128, 128], fp32)
        nc.sync.dma_start(out=dwk, in_=dwk_in[:, :])
        ewk = singles.tile([128, 128], fp32)
        nc.sync.dma_start(out=ewk, in_=ewk_in[:, :])
        ewkT = singles.tile([128, 128], fp32)
        nc.sync.dma_start(out=ewkT, in_=ewkT_in[:, :])

        gb0 = singles.tile([128, 1], fp32)
        nc.sync.dma_start(out=gb0, in_=gate_b_in[0:128, :])
        gb1 = singles.tile([128, 1], fp32)
        nc.sync.dma_start(out=gb1, in_=gate_b_in[128:256, :])
        cb = singles.tile([128, 1], fp32)
        nc.sync.dma_start(out=cb, in_=cand_b_in[:, :])
        dwb = singles.tile([128, 1], fp32)
        nc.sync.dma_start(out=dwb, in_=dwb_in[:, :])
        ewb = singles.tile([128, 1], fp32)
        nc.sync.dma_start(out=ewb, in_=ewb_in[:, :])
        vkT = singles.tile([128, 1], fp32)
        nc.sync.dma_start(out=vkT, in_=vk_in[:, :])

        if use_fcb:
            fcb_sb = singles.tile([1, V], fp32)
            nc.sync.dma_start(out=fcb_sb, in_=fcb_in[:, :])
        if use_mask:
            maskb_sb = singles.tile([1, BC * T], fp32)
            nc.sync.dma_start(out=maskb_sb, in_=maskb_in[:, :])

        toks_sb = singles.tile([128, 2], mybir.dt.int32)
        nc.sync.dma_start(out=toks_sb, in_=toks_in[:, :])

        # ---- enc loads: raw [t,h] tiles and transposed [h,t] ------------
        enc_sb, encT_sb = [], []
        for b in range(BC):
            e_t = enc_pool.tile([128, 16, 128], fp32, name=f"enc_{b}", tag=f"enc_{b}")
            nc.sync.dma_start(out=e_t, in_=enc_in[b].rearrange("(k p) h -> p k h", p=128))
            enc_sb.append(e_t)
            eT = enc_pool.tile([128, T], f32r, name=f"encT_{b}", tag=f"encT_{b}")
            nc.sync.dma_start(out=eT, in_=encT_in[b].bitcast(f32r))
            encT_sb.append(eT)

        # ---- embedding gather + transpose -> xT [E, (l,b)] -------------
        xT = singles.tile([128, L * BC], f32r)      # columns l*8+b, local batches
        for g in range(2):
            x_g = gru_pool.tile([128, 128], fp32, tag="x_g")
            nc.gpsimd.indirect_dma_start(
                out=x_g,
                out_offset=None,
                in_=embed_in[:, :],
                in_offset=bass.IndirectOffsetOnAxis(ap=toks_sb[:, g:g + 1], axis=0),
            )
            x_g2 = gru_pool.tile([128, 128], fp32, tag="x_g2")
            nc.vector.tensor_copy(out=x_g2, in_=x_g)
            xg_ps = ps1.tile([128, 128], fp32, tag="ps1")
            nc.tensor.transpose(out=xg_ps, in_=x_g2, identity=ident)
            nc.vector.tensor_copy(out=xT[:, g * 128:(g + 1) * 128], in_=xg_ps)

        # ---- GRU recurrence over L steps (local batches) ----------------
        # x-projections accumulate directly in PSUM (no h dependence, so the
        # scheduler runs them ahead); biases applied on the ACT engine.
        HallT = singles.tile([128, L * BC], f32r)   # h_l^T columns l*8+b
        h0 = singles.tile([128, BC], fp32)
        nc.vector.memset(h0, 0.0)
        hT = h0.bitcast(f32r)
        for l in range(L):
            xs = xT[:, l * BC:(l + 1) * BC]
            g_ps = ps1.tile([128, 2 * BC], fp32, tag="ps1", name=f"g_ps{l}")
            nc.tensor.matmul(out=g_ps[:, 0:BC], lhsT=gkx[:, 0:128], rhs=xs, start=True, stop=False)
            nc.tensor.matmul(out=g_ps[:, 0:BC], lhsT=gkh[:, 0:128], rhs=hT, start=False, stop=True)
            nc.tensor.matmul(out=g_ps[:, BC:2 * BC], lhsT=gkx[:, 128:256], rhs=xs, start=True, stop=False)
            nc.tensor.matmul(out=g_ps[:, BC:2 * BC], lhsT=gkh[:, 128:256], rhs=hT, start=False, stop=True)
            r_sb = gru_pool.tile([128, BC], fp32, tag="r_sb")
            u_sb = gru_pool.tile([128, BC], fp32, tag="u_sb")
            nc.scalar.activation(out=r_sb, in_=g_ps[:, 0:BC], func=AF.Sigmoid, bias=gb0)
            nc.scalar.activation(out=u_sb, in_=g_ps[:, BC:2 * BC], func=AF.Sigmoid, bias=gb1)
            rh = gru_pool.tile([128, BC], f32r, tag="rh")
            nc.vector.tensor_mul(out=rh, in0=r_sb, in1=hT)
            c_ps = ps1.tile([128, BC], fp32, tag="ps1", name=f"c_ps{l}")
            nc.tensor.matmul(out=c_ps, lhsT=ckx, rhs=xs, start=True, stop=False)
            nc.tensor.matmul(out=c_ps, lhsT=ckh, rhs=rh, start=False, stop=True)
            c_sb = gru_pool.tile([128, BC], fp32, tag="c_sb")
            nc.scalar.activation(out=c_sb, in_=c_ps, func=AF.Tanh, bias=cb)
            d_sb = gru_pool.tile([128, BC], fp32, tag="d_sb")
            nc.vector.tensor_sub(out=d_sb, in0=hT, in1=c_sb)        # h - c
            nc.vector.tensor_mul(out=d_sb, in0=u_sb, in1=d_sb)
            hcol = HallT[:, l * BC:(l + 1) * BC]
            nc.vector.tensor_add(out=hcol, in0=c_sb, in1=d_sb)      # c + u*(h-c)
            hT = hcol

        Hall3 = HallT.rearrange("p (l b) -> p l b", b=BC)

        # ---- attention, two groups of 4 batches -------------------------
        # wproj for all 8 batches in one batched chain:
        #   wpT_all[h', (l,b)] = encW_k @ ((dec_feat + decW_b) * V_k)^T
        dec_ps = ps1.tile([128, 256], fp32, tag="ps1")
        nc.tensor.matmul(out=dec_ps, lhsT=dwk, rhs=HallT.bitcast(fp32), start=True, stop=True)
        wT_all = singles.tile([128, 256], fp32)
        nc.vector.tensor_scalar(out=wT_all, in0=dec_ps, scalar1=dwb, scalar2=vkT,
                                op0=ALU.add, op1=ALU.mult)
        wp_ps = ps1.tile([128, 256], fp32, tag="ps1")
        nc.tensor.matmul(out=wp_ps, lhsT=ewkT, rhs=wT_all, start=True, stop=True)
        wpT_all = singles.tile([128, 256], f32r)
        nc.vector.tensor_copy(out=wpT_all, in_=wp_ps)
        wpT3 = wpT_all.rearrange("p (l b) -> p l b", b=BC)

        ctxT_all = singles.tile([128, ROWS], f32r)    # columns b_local*32+l
        for g01 in range(2):
            sc_ps = ps_score.tile([128, T], fp32, tag="score", name=f"sc{g01}")
            mx4 = gru_pool.tile([128, 4], fp32, tag="mx4")
            for c in range(4):
                for j in range(4):
                    b = g01 * 4 + j
                    nc.tensor.matmul(out=sc_ps[32 * j:32 * (j + 1), c * 512:(c + 1) * 512],
                                     lhsT=wpT3[:, :, b].bitcast(fp32),
                                     rhs=encT_sb[b][:, c * 512:(c + 1) * 512].bitcast(fp32),
                                     start=True, stop=not use_mask,
                                     tile_position=(0, 32 * j))
                    if use_mask:
                        nc.tensor.matmul(out=sc_ps[32 * j:32 * (j + 1), c * 512:(c + 1) * 512],
                                         lhsT=ones_row[:, 0:32],
                                         rhs=maskb_sb[:, b * T + c * 512: b * T + (c + 1) * 512],
                                         start=False, stop=True,
                                         tile_position=(0, 32 * j))
                nc.vector.tensor_reduce(out=mx4[:, c:c + 1], in_=sc_ps[:, c * 512:(c + 1) * 512],
                                        axis=AX.X, op=ALU.max)
            nmx = gru_pool.tile([128, 1], fp32, tag="nmx")
            nc.vector.tensor_reduce(out=nmx, in_=mx4, axis=AX.X, op=ALU.max, negate=True)
            attn_sb = attn_pool.tile([128, T], fp32, tag="attn")
            sume = gru_pool.tile([128, 1], fp32, tag="sume")
            nc.scalar.activation(out=attn_sb, in_=sc_ps, func=AF.Exp, bias=nmx,
                                 accum_out=sume)
            rs = gru_pool.tile([128, 1], fp32, tag="rs")
            nc.vector.reciprocal(out=rs, in_=sume)
            nc.vector.tensor_scalar_mul(attn_sb, attn_sb, rs)
            nc.sync.dma_start(out=attns_out[g01 * 128:(g01 + 1) * 128, :], in_=attn_sb)

            # transpose attn -> [T, (4b,32l)] tiles
            attnT = attnT_pool.tile([128, 16, 128], fp32, tag="attnT")
            for k in range(16):
                at_ps = ps1.tile([128, 128], fp32, tag="ps1")
                nc.tensor.transpose(out=at_ps, in_=attn_sb[:, k * 128:(k + 1) * 128], identity=ident)
                nc.vector.tensor_copy(out=attnT[:, k, :], in_=at_ps)

            # ctx_raw accumulation over t tiles
            cx2_ps = ps1.tile([128, 128], fp32, tag="ps1", name=f"cx2_{g01}")
            for j in range(4):
                b = g01 * 4 + j
                for k in range(16):
                    nc.tensor.matmul(out=cx2_ps[32 * j:32 * (j + 1), :],
                                     lhsT=attnT[:, k, 32 * j:32 * (j + 1)],
                                     rhs=enc_sb[b][:, k, :],
                                     start=(k == 0), stop=(k == 15),
                                     tile_position=(0, 32 * j))
            cxr = gru_pool.tile([128, 128], fp32, tag="cxr")
            nc.vector.tensor_copy(out=cxr, in_=cx2_ps)
            cxT_ps = ps1.tile([128, 128], fp32, tag="ps1", name=f"cxT{g01}")
            nc.tensor.transpose(out=cxT_ps, in_=cxr, identity=ident)
            cxrT = gru_pool.tile([128, 128], fp32, tag="cxrT")
            nc.vector.tensor_copy(out=cxrT, in_=cxT_ps)
            cf_ps = ps1.tile([128, 128], fp32, tag="ps1", name=f"cf{g01}")
            nc.tensor.matmul(out=cf_ps, lhsT=ewk, rhs=cxrT, start=True, stop=True)
            nc.vector.tensor_scalar_add(ctxT_all[:, g01 * 128:(g01 + 1) * 128], cf_ps, ewb)

        # ---- fc to vocab -------------------------------------------------
        for fi in range(FC_NB):
            fck_sb = fck_pool.tile([128, FC_CH], f32r, tag="fck")
            nc.sync.dma_start(out=fck_sb, in_=fck_in[:, fi * FC_CH:(fi + 1) * FC_CH].bitcast(f32r))
            for mi in range(2):
                lg_sb = lg_pool.tile([128, FC_CH], fp32, tag="lg")
                for cc in range(4):
                    n0 = cc * 400
                    fc_ps = ps1.tile([128, 400], fp32, tag="ps1", name=f"fc{fi}_{mi}_{cc}")
                    nc.tensor.matmul(out=fc_ps,
                                     lhsT=ctxT_all[:, mi * 128:(mi + 1) * 128],
                                     rhs=fck_sb[:, n0:n0 + 400],
                                     start=True, stop=not use_fcb)
                    if use_fcb:
                        nc.tensor.matmul(out=fc_ps, lhsT=ones_row,
                                         rhs=fcb_sb[:, fi * FC_CH + n0: fi * FC_CH + n0 + 400],
                                         start=False, stop=True)
                    nc.vector.tensor_copy(out=lg_sb[:, n0:n0 + 400], in_=fc_ps)
                nc.sync.dma_start(out=logits_out[mi * 128:(mi + 1) * 128, fi * FC_CH:(fi + 1) * FC_CH],
                                  in_=lg_sb)

    nc.finalize()   # Bacc.compile(): splits multi-waits via event semaphores
    return nc


def _get_nc(use_mask: bool, use_fcb: bool):
    key = (use_mask, use_fcb)
    if key not in _cache:
        _cache[key] = _build_nc(use_mask, use_fcb)
    return _cache[key]


def _host_prep(inputs):
    ti = np.asarray(inputs["target_input"])
    toks = np.concatenate(
        [np.full((B, 1), SOS_ID, np.int64), ti[:, :-1].astype(np.int64)], axis=1)
    mask = np.asarray(inputs["weight_mask"]).astype(np.float32)
    maskb = (mask - 1.0) * 1e30                        # 0 valid / -1e30 masked
    return toks, maskb


def kernel(**inputs):
    return _run(inputs, trace=False)[0]


def _run(inputs, trace=False):
    from concourse.bass_utils import run_bass_kernel_spmd

    toks, maskb = _host_prep(inputs)
    use_mask = bool(np.any(maskb != 0.0))
    fc_b = np.asarray(inputs["fc_b"], np.float32)
    use_fcb = bool(np.any(fc_b != 0.0))

    nc = _get_nc(use_mask, use_fcb)

    enc = np.ascontiguousarray(np.asarray(inputs["enc_output"], np.float32))
    encT = np.ascontiguousarray(enc.transpose(0, 2, 1))
    rep = dict(
        embed=np.asarray(inputs["embed"], np.float32),
        gate_k=np.asarray(inputs["gate_k"], np.float32),
        gate_b=np.asarray(inputs["gate_b"], np.float32).reshape(2 * H, 1),
        cand_k=np.asarray(inputs["cand_k"], np.float32),
        cand_b=np.asarray(inputs["cand_b"], np.float32).reshape(H, 1),
        decW_k=np.asarray(inputs["decW_k"], np.float32),
        decW_b=np.asarray(inputs["decW_b"], np.float32).reshape(H, 1),
        encW_k=np.asarray(inputs["encW_k"], np.float32),
        encW_kT=np.ascontiguousarray(np.asarray(inputs["encW_k"], np.float32).T),
        encW_b=np.asarray(inputs["encW_b"], np.float32).reshape(H, 1),
        V_k=np.asarray(inputs["V_k"], np.float32).reshape(H, 1),
        fc_k=np.asarray(inputs["fc_k"], np.float32),
    )
    if use_fcb:
        rep["fc_b"] = fc_b.reshape(1, V)

    in_maps = []
    for ci in range(NC_N):
        m = dict(rep)
        m["enc_in"] = enc[ci * BC:(ci + 1) * BC]
        m["encT_in"] = encT[ci * BC:(ci + 1) * BC]
        # rows r = l*BC + b_local, stored [128, 2] with r = g*128 + p
        tk = toks[ci * BC:(ci + 1) * BC]               # [BC, L]
        tk_lb = tk.T.reshape(-1)                       # (l, b) order
        m["toks"] = np.ascontiguousarray(tk_lb.reshape(2, 128).T.astype(np.int32))
        if use_mask:
            m["maskb"] = np.ascontiguousarray(
                maskb[ci * BC:(ci + 1) * BC].reshape(1, BC * T))
        in_maps.append(m)

    res = run_bass_kernel_spmd(nc, in_maps, list(range(NC_N)), trace=trace)
    logits = np.concatenate([r["logits"].reshape(BC, L, V) for r in res.results], axis=0)
    attns = np.concatenate([r["attns"].reshape(BC, L, T) for r in res.results], axis=0)
    return (logits, attns), res
